# revision 16
# baseline (speedup 1.0000x reference)
"""DeepSeekMoE (T=4096, H=1024, I=2048, E=8 routed top-2 + 1 shared) on 8 TRN2 NeuronCores.

Strategy (expert-parallel + replicated-data hybrid):
  - Each core c owns routed expert c (weights sharded over cores).
  - Router runs data-parallel (each core routes its 512 tokens, exact-fp32 via
    bf16 hi/lo 3-product matmuls), results AllGather'd (tiny).
  - Shared expert is I-sliced (each core computes a 256-wide slice of the
    intermediate dim for ALL tokens), producing a dense [T,H] partial per core.
  - Each core compacts the token list routed to its expert (prefix-scan +
    triangular-ones matmul + indirect-DMA scatter), gathers those token rows,
    computes the expert MLP on a fixed capacity batch, scales by gates and
    scatter-adds into its [T,H] partial.
  - ReduceScatter(add) over the 8 partials yields each core's final 512-token
    slice; host just concatenates.

All MLP matmuls run in bf16 (fp32 PSUM accumulation); the router is exact to
fp32 working precision so top-2 selection matches the fp32 reference.
"""

from contextlib import ExitStack

import numpy as np
import ml_dtypes

import concourse.bass as bass
import concourse.mybir as mybir
from concourse.tile import TileContext
from concourse.masks import make_identity

BF = ml_dtypes.bfloat16

T = 4096          # tokens
H = 1024          # hidden
I = 2048          # intermediate
E = 8             # routed experts
NCORE = 8
TPC = T // NCORE  # tokens per core (512)
ISL = I // NCORE  # shared-expert I-slice per core (256)
CAP = 1280        # per-expert token capacity (seed-0 max count is 1076)
NTT = TPC // 128  # local token tiles (4)
NHB = H // 128    # hidden 128-blocks (8)
NIT = I // 128    # intermediate 128-blocks (16)
NCT = CAP // 128  # capacity tiles (10)
BIGPOS = 60000.0  # out-of-bounds scatter position for unassigned tokens

FP32 = mybir.dt.float32
BF16 = mybir.dt.bfloat16
I32 = mybir.dt.int32
U32 = mybir.dt.uint32


def ts(i, s):
    return slice(i * s, (i + 1) * s)


def split_multiwait(nc, max_waits=1):
    """This container's walrus build rejects instructions carrying more than
    one fused semaphore wait ("Too many sync wait commands"). Offload extra
    waits onto standalone EventSemaphore instructions ahead of the owner —
    identical semantics (the sequencer blocks either way)."""
    n_split = 0
    for fn in nc.m.functions:
        for blk in fn.blocks:
            out = []
            for ins in blk.instructions:
                si = ins.sync_info
                if si is not None and si.on_wait and len(si.on_wait) > max_waits:
                    waits = list(si.on_wait)
                    for i, w in enumerate(waits[max_waits:]):
                        ev = mybir.InstEventSemaphore(
                            name=f"{ins.name}-evw{i}",
                            engine=ins.engine,
                            sync_info=mybir.SyncInfo(on_wait=[w], on_update=[]),
                        )
                        out.append(ev)
                        n_split += 1
                    si.on_wait = waits[:max_waits]
                out.append(ins)
            blk.instructions = out
    return n_split


def build_module(debug=False, split=True):
    nc = bass.Bass(num_devices=NCORE)

    # ---------------- external inputs (per-core host-prepared) ----------------
    def inp(name, shape, dtype):
        return nc.declare_dram_parameter(name, list(shape), dtype, isOutput=False)

    x_rows = inp("x_rows", (T, H), BF16)          # token-major x (gather source)
    xT = inp("xT", (H, T), BF16)                  # x transposed (shared-expert rhs)
    xTl_h = inp("xTl_h", (H, TPC), BF16)          # local x.T hi (router lhsT)
    xTl_l = inp("xTl_l", (H, TPC), BF16)          # local x.T lo
    rwT_h = inp("rwT_h", (H, E), BF16)            # router w.T hi
    rwT_l = inp("rwT_l", (H, E), BF16)
    bias_bc = inp("bias_bc", (128, E), FP32)      # routing bias broadcast to 128 rows
    wgT = inp("wgT", (H, I), BF16)                # this core's expert gate w.T
    wuT = inp("wuT", (H, I), BF16)
    wdT = inp("wdT", (I, H), BF16)
    sgT = inp("sgT", (H, ISL), BF16)              # shared gate w slice .T
    suT = inp("suT", (H, ISL), BF16)
    sdT = inp("sdT", (ISL, H), BF16)              # shared down w slice .T
    cvec = inp("cvec", (128, 1), FP32)            # core id replicated
    ut_ones = inp("ut_ones", (128, 128), BF16)    # strict upper-triangular ones

    out_ext = nc.declare_dram_parameter("out", [TPC, H], FP32, isOutput=True)
    if debug:
        dbg_rt = nc.declare_dram_parameter("dbg_rt", [NCORE, 128, 16], FP32, isOutput=True)
        dbg_cmp = nc.declare_dram_parameter("dbg_cmp", [CAP, 2], FP32, isOutput=True)
        dbg_partial = nc.declare_dram_parameter("dbg_partial", [T, H], BF16, isOutput=True)
        dbg_xgt = nc.declare_dram_parameter("dbg_xgt", [128, NHB, CAP], BF16, isOutput=True)

    with TileContext(nc) as tc, ExitStack() as ctx:
        sb = ctx.enter_context(tc.tile_pool(name="sb", bufs=1))
        sb2 = ctx.enter_context(tc.tile_pool(name="sb2", bufs=2))
        sb3 = ctx.enter_context(tc.tile_pool(name="sb3", bufs=3))
        ps_big = ctx.enter_context(tc.tile_pool(name="ps_big", bufs=4, space="PSUM"))
        ps_sm = ctx.enter_context(tc.tile_pool(name="ps_sm", bufs=2, space="PSUM"))
        dram = ctx.enter_context(tc.tile_pool(name="dram", bufs=1, space="DRAM"))

        ident = sb.tile([128, 128], BF16, name="ident")
        make_identity(nc, ident[:])

        # ------------------------------------------------------------------
        # Phase R: router on local 512 tokens.
        # logits[128t, 8e] = sum_hb (xh+xl).T @ (wh+wl)  (3 products, fp32 acc)
        # ------------------------------------------------------------------
        xtlh_sb = sb.tile([128, NHB, TPC], BF16, name="xtlh_sb")
        xtll_sb = sb.tile([128, NHB, TPC], BF16, name="xtll_sb")
        rwh_sb = sb.tile([128, NHB, E], BF16, name="rwh_sb")
        rwl_sb = sb.tile([128, NHB, E], BF16, name="rwl_sb")
        bias_sb = sb.tile([128, E], FP32, name="bias_sb")
        nc.sync.dma_start(out=xtlh_sb[:], in_=xTl_h.rearrange("(b p) t -> p b t", p=128))
        nc.sync.dma_start(out=xtll_sb[:], in_=xTl_l.rearrange("(b p) t -> p b t", p=128))
        nc.sync.dma_start(out=rwh_sb[:], in_=rwT_h.rearrange("(b p) e -> p b e", p=128))
        nc.sync.dma_start(out=rwl_sb[:], in_=rwT_l.rearrange("(b p) e -> p b e", p=128))
        nc.sync.dma_start(out=bias_sb[:], in_=bias_bc[:])

        rtloc = sb.tile([128, NTT, 4], FP32, name="rtloc")  # (i1, i2, g1, g2) per tile
        for tt in range(NTT):
            ps_r = ps_sm.tile([128, E], FP32, name="ps_r", tag="ps_sm")
            k = 0
            pairs = [(xtlh_sb, rwh_sb), (xtlh_sb, rwl_sb), (xtll_sb, rwh_sb)]
            nmm = len(pairs) * NHB
            for xs, ws in pairs:
                for hb in range(NHB):
                    nc.tensor.matmul(
                        out=ps_r[:],
                        lhsT=xs[:, hb, ts(tt, 128)],
                        rhs=ws[:, hb, :],
                        start=(k == 0),
                        stop=(k == nmm - 1),
                    )
                    k += 1
            logit = sb2.tile([128, E], FP32, name="logit")
            nc.vector.tensor_add(out=logit[:], in0=ps_r[:], in1=bias_sb[:])
            vals = sb2.tile([128, 8], FP32, name="vals")
            idxs = sb2.tile([128, 8], U32, name="idxs")
            nc.vector.max(out=vals[:], in_=logit[:])
            nc.vector.max_index(out=idxs[:], in_max=vals[:], in_values=logit[:])
            p12 = sb2.tile([128, 2], FP32, name="p12")
            nc.scalar.activation(p12[:], vals[:, 0:2], mybir.ActivationFunctionType.Sigmoid)
            psum12 = sb2.tile([128, 1], FP32, name="psum12")
            nc.vector.tensor_add(out=psum12[:], in0=p12[:, 0:1], in1=p12[:, 1:2])
            rinv = sb2.tile([128, 1], FP32, name="rinv")
            nc.vector.reciprocal(out=rinv[:], in_=psum12[:])
            # pack (i1, i2, g1, g2)
            nc.vector.tensor_copy(rtloc[:, tt, 0:2], idxs[:, 0:2])
            nc.vector.tensor_scalar_mul(rtloc[:, tt, 2:4], p12[:], rinv[:])

        rt_local = dram.tile([128, NTT * 4], FP32, name="rt_local")
        rt_all = dram.tile([NCORE, 128, NTT * 4], FP32, name="rt_all", addr_space="Shared")
        nc.sync.dma_start(out=rt_local[:], in_=rtloc[:].rearrange("p t f -> p (t f)"))
        nc.gpsimd.collective_compute(
            "AllGather",
            mybir.AluOpType.bypass,
            replica_groups=[list(range(NCORE))],
            ins=[rt_local[:]],
            outs=[rt_all[:]],
        )

        # ------------------------------------------------------------------
        # Phase S: shared expert, I-slice [ISL] for ALL T tokens.
        # partial[T, H] (bf16) = (silu(x @ sg.T) * (x @ su.T))[:, slice] @ sd.T[slice]
        # ------------------------------------------------------------------
        partial = dram.tile([T, H], BF16, name="partial")
        sg_sb = sb.tile([128, NHB, ISL], BF16, name="sg_sb")
        su_sb = sb.tile([128, NHB, ISL], BF16, name="su_sb")
        sd_sb = sb.tile([128, ISL // 128, H], BF16, name="sd_sb")
        nc.sync.dma_start(out=sg_sb[:], in_=sgT.rearrange("(b p) i -> p b i", p=128))
        nc.sync.dma_start(out=su_sb[:], in_=suT.rearrange("(b p) i -> p b i", p=128))
        nc.sync.dma_start(out=sd_sb[:], in_=sdT.rearrange("(b p) h -> p b h", p=128))

        NSC = T // 512  # 8 chunks of 512 tokens
        for ch in range(NSC):
            xt_sb = sb3.tile([128, NHB, 512], BF16, name="xt_sb", tag="xt_sb")
            nc.sync.dma_start(
                out=xt_sb[:], in_=xT[:, ts(ch, 512)].rearrange("(b p) t -> p b t", p=128)
            )
            hts = sb2.tile([128, ISL // 128, 512], BF16, name="hts", tag="hts")
            for it in range(ISL // 128):
                ps_g = ps_big.tile([128, 512], FP32, name="ps_g", tag="ps_big")
                ps_u = ps_big.tile([128, 512], FP32, name="ps_u", tag="ps_big")
                for hb in range(NHB):
                    nc.tensor.matmul(
                        out=ps_g[:], lhsT=sg_sb[:, hb, ts(it, 128)], rhs=xt_sb[:, hb, :],
                        start=(hb == 0), stop=(hb == NHB - 1),
                    )
                for hb in range(NHB):
                    nc.tensor.matmul(
                        out=ps_u[:], lhsT=su_sb[:, hb, ts(it, 128)], rhs=xt_sb[:, hb, :],
                        start=(hb == 0), stop=(hb == NHB - 1),
                    )
                sil = sb2.tile([128, 512], FP32, name="sil", tag="sil")
                nc.scalar.activation(sil[:], ps_g[:], mybir.ActivationFunctionType.Sigmoid)
                nc.vector.tensor_mul(out=sil[:], in0=sil[:], in1=ps_g[:])
                nc.vector.tensor_mul(out=hts[:, it, :], in0=sil[:], in1=ps_u[:])
            # down: out[tok, H] with lhsT = hts slices (I on K), rhs = sd
            for mt in range(4):  # 128-token subtiles of this chunk
                orow = sb2.tile([128, H], BF16, name="orow", tag="orow")
                for nch in range(H // 512):
                    ps_d = ps_big.tile([128, 512], FP32, name="ps_d", tag="ps_big")
                    for it in range(ISL // 128):
                        nc.tensor.matmul(
                            out=ps_d[:],
                            lhsT=hts[:, it, ts(mt, 128)],
                            rhs=sd_sb[:, it, ts(nch, 512)],
                            start=(it == 0),
                            stop=(it == ISL // 128 - 1),
                        )
                    nc.vector.tensor_copy(orow[:, ts(nch, 512)], ps_d[:])
                nc.sync.dma_start(
                    out=partial[ch * 512 + mt * 128 : ch * 512 + (mt + 1) * 128, :],
                    in_=orow[:],
                )

        # ------------------------------------------------------------------
        # Phase C: routing bookkeeping over all T tokens (after AllGather).
        # Build compact (token, gate) list for THIS core's expert.
        # ------------------------------------------------------------------
        NJ = NCORE * NTT  # 32 columns; col j=(r*4+tt), token = 512*(j//4)+128*(j%4)+p
        rt_sb = sb.tile([128, NJ, 4], FP32, name="rt_sb")
        nc.sync.dma_start(
            out=rt_sb[:].rearrange("p (r t) f -> p r t f", r=NCORE),
            in_=rt_all.rearrange("r p (t f) -> p r t f", f=4),
        )
        m1 = sb.tile([128, NJ], FP32, name="m1")
        m2 = sb.tile([128, NJ], FP32, name="m2")
        mask = sb.tile([128, NJ], FP32, name="mask")
        gate_c = sb.tile([128, NJ], FP32, name="gate_c")
        cvec_sb = sb.tile([128, 1], FP32, name="cvec_sb")
        nc.sync.dma_start(out=cvec_sb[:], in_=cvec[:])
        nc.vector.tensor_scalar(m1[:], rt_sb[:, :, 0], cvec_sb[:], None, op0=mybir.AluOpType.is_equal)
        nc.vector.tensor_scalar(m2[:], rt_sb[:, :, 1], cvec_sb[:], None, op0=mybir.AluOpType.is_equal)
        nc.vector.tensor_add(out=mask[:], in0=m1[:], in1=m2[:])
        tmp1 = sb.tile([128, NJ], FP32, name="tmp1")
        nc.vector.tensor_mul(out=tmp1[:], in0=m1[:], in1=rt_sb[:, :, 2])
        nc.vector.tensor_mul(out=gate_c[:], in0=m2[:], in1=rt_sb[:, :, 3])
        nc.vector.tensor_add(out=gate_c[:], in0=gate_c[:], in1=tmp1[:])

        zeros = sb.tile([128, NJ], FP32, name="zeros")
        nc.vector.memset(zeros[:], 0.0)
        rowcum = sb.tile([128, NJ], FP32, name="rowcum")
        nc.vector.tensor_tensor_scan(
            out=rowcum[:], data0=mask[:], data1=zeros[:], initial=0.0,
            op0=mybir.AluOpType.add, op1=mybir.AluOpType.add,
        )
        rowtot_bf = sb.tile([128, 1], BF16, name="rowtot_bf")
        nc.vector.tensor_copy(rowtot_bf[:], rowcum[:, NJ - 1 : NJ])
        ut_sb = sb.tile([128, 128], BF16, name="ut_sb")
        nc.sync.dma_start(out=ut_sb[:], in_=ut_ones[:])
        ps_cum = ps_sm.tile([128, 1], FP32, name="ps_cum", tag="ps_sm")
        nc.tensor.matmul(out=ps_cum[:], lhsT=ut_sb[:], rhs=rowtot_bf[:], start=True, stop=True)
        base = sb.tile([128, 1], FP32, name="base")
        nc.vector.tensor_copy(base[:], ps_cum[:])

        pos = sb.tile([128, NJ], FP32, name="pos")
        nc.vector.tensor_sub(out=pos[:], in0=rowcum[:], in1=mask[:])
        nc.vector.tensor_scalar_add(pos[:], pos[:], base[:])
        # unassigned -> BIGPOS (dropped by scatter bounds check)
        notm = sb.tile([128, NJ], FP32, name="notm")
        nc.vector.tensor_scalar(notm[:], mask[:], -BIGPOS, BIGPOS,
                                op0=mybir.AluOpType.mult, op1=mybir.AluOpType.add)
        nc.vector.tensor_add(out=pos[:], in0=pos[:], in1=notm[:])
        upos = sb.tile([128, NJ], I32, name="upos")
        nc.vector.tensor_copy(upos[:], pos[:])

        tok_iota = sb.tile([128, NCORE, NTT], I32, name="tok_iota")
        nc.gpsimd.iota(tok_iota[:], pattern=[[TPC, NCORE], [128, NTT]], base=0,
                       channel_multiplier=1)
        rec = sb.tile([128, NJ, 2], FP32, name="rec")
        nc.vector.tensor_copy(rec[:, :, 0], tok_iota[:].rearrange("p a b -> p (a b)"))
        nc.vector.tensor_copy(rec[:, :, 1], gate_c[:])

        cmp_d = dram.tile([CAP, 2], FP32, name="cmp_d")
        zrow = sb.tile([128, CAP * 2 // 128], FP32, name="zrow")
        nc.vector.memset(zrow[:], 0.0)
        nc.sync.dma_start(out=cmp_d.rearrange("(p t) f -> p (t f)", p=128), in_=zrow[:])
        # HW indirect DMA honors one offset per partition: scatter column-wise.
        for j in range(NJ):
            nc.gpsimd.indirect_dma_start(
                out=cmp_d[:],
                out_offset=bass.IndirectOffsetOnAxis(ap=upos[:, j : j + 1], axis=0),
                in_=rec[:, j, :],
                in_offset=None,
                bounds_check=CAP - 1,
                oob_is_err=False,
            )
        # read back compact list: slot s = t*128 + p  ->  [p, t]
        cmp_sb = sb.tile([128, NCT, 2], FP32, name="cmp_sb")
        nc.sync.dma_start(out=cmp_sb[:], in_=cmp_d.rearrange("(t p) f -> p t f", p=128))
        tok_i = sb.tile([128, NCT], I32, name="tok_i")
        nc.vector.tensor_copy(tok_i[:], cmp_sb[:, :, 0])

        # ------------------------------------------------------------------
        # Phase G: gather x rows for this expert's tokens; transpose to [H, cap].
        # ------------------------------------------------------------------
        xgT = sb.tile([128, NHB, CAP], BF16, name="xgT")
        for ct in range(NCT):
            xg = sb2.tile([128, H], BF16, name="xg", tag="xg")
            nc.gpsimd.indirect_dma_start(
                out=xg[:],
                out_offset=None,
                in_=x_rows[:],
                in_offset=bass.IndirectOffsetOnAxis(ap=tok_i[:, ct : ct + 1], axis=0),
            )
            for hb in range(NHB):
                ps_t = ps_sm.tile([128, 128], BF16, name="ps_t", tag="ps_sm")
                nc.tensor.transpose(out=ps_t[:], in_=xg[:, ts(hb, 128)], identity=ident[:])
                nc.vector.tensor_copy(xgT[:, hb, ts(ct, 128)], ps_t[:])

        # ------------------------------------------------------------------
        # Phase E: routed expert MLP on the capacity batch.
        # ------------------------------------------------------------------
        hT = sb.tile([128, NIT, CAP], BF16, name="hT")
        ECH = [(0, 512), (512, 512), (1024, CAP - 1024)]  # cap chunks
        for it in range(NIT):
            wg_sb = sb2.tile([128, NHB, 128], BF16, name="wg_sb", tag="wg_sb")
            wu_sb = sb2.tile([128, NHB, 128], BF16, name="wu_sb", tag="wu_sb")
            nc.sync.dma_start(
                out=wg_sb[:], in_=wgT[:, ts(it, 128)].rearrange("(b p) i -> p b i", p=128)
            )
            nc.sync.dma_start(
                out=wu_sb[:], in_=wuT[:, ts(it, 128)].rearrange("(b p) i -> p b i", p=128)
            )
            for c0, cn in ECH:
                ps_g = ps_big.tile([128, 512], FP32, name="ps_g", tag="ps_big")
                ps_u = ps_big.tile([128, 512], FP32, name="ps_u", tag="ps_big")
                for hb in range(NHB):
                    nc.tensor.matmul(
                        out=ps_g[:, :cn], lhsT=wg_sb[:, hb, :], rhs=xgT[:, hb, c0 : c0 + cn],
                        start=(hb == 0), stop=(hb == NHB - 1),
                    )
                for hb in range(NHB):
                    nc.tensor.matmul(
                        out=ps_u[:, :cn], lhsT=wu_sb[:, hb, :], rhs=xgT[:, hb, c0 : c0 + cn],
                        start=(hb == 0), stop=(hb == NHB - 1),
                    )
                sil = sb2.tile([128, 512], FP32, name="sil", tag="sil")
                nc.scalar.activation(sil[:, :cn], ps_g[:, :cn], mybir.ActivationFunctionType.Sigmoid)
                nc.vector.tensor_mul(out=sil[:, :cn], in0=sil[:, :cn], in1=ps_g[:, :cn])
                nc.vector.tensor_mul(out=hT[:, it, c0 : c0 + cn], in0=sil[:, :cn], in1=ps_u[:, :cn])

        wd_sb = sb.tile([128, NIT, H], BF16, name="wd_sb")
        nc.sync.dma_start(out=wd_sb[:], in_=wdT.rearrange("(b p) h -> p b h", p=128))

        # ------------------------------------------------------------------
        # Phase A: down-proj per capacity tile, scale by gate, scatter-add.
        # ------------------------------------------------------------------
        for ct in range(NCT):
            yrow = sb2.tile([128, H], BF16, name="yrow", tag="yrow")
            for nch in range(H // 512):
                ps_d = ps_big.tile([128, 512], FP32, name="ps_d", tag="ps_big")
                for it in range(NIT):
                    nc.tensor.matmul(
                        out=ps_d[:],
                        lhsT=hT[:, it, ts(ct, 128)],
                        rhs=wd_sb[:, it, ts(nch, 512)],
                        start=(it == 0),
                        stop=(it == NIT - 1),
                    )
                nc.vector.tensor_scalar_mul(yrow[:, ts(nch, 512)], ps_d[:], cmp_sb[:, ct, 1:2])
            nc.gpsimd.indirect_dma_start(
                out=partial[:],
                out_offset=bass.IndirectOffsetOnAxis(ap=tok_i[:, ct : ct + 1], axis=0),
                in_=yrow[:],
                in_offset=None,
                compute_op=mybir.AluOpType.add,
                bounds_check=T - 1,
                oob_is_err=False,
            )

        if debug:
            nc.sync.dma_start(out=dbg_rt[:], in_=rt_all[:])
            nc.sync.dma_start(out=dbg_cmp[:], in_=cmp_d[:])
            nc.sync.dma_start(out=dbg_partial[:], in_=partial[:])
            nc.sync.dma_start(out=dbg_xgt[:], in_=xgT[:])

        # ------------------------------------------------------------------
        # Phase RS: ReduceScatter over the 8 partials -> this core's 512 rows.
        # ------------------------------------------------------------------
        rs_out = dram.tile([TPC, H], BF16, name="rs_out")
        nc.gpsimd.collective_compute(
            "ReduceScatter",
            mybir.AluOpType.add,
            replica_groups=[list(range(NCORE))],
            ins=[partial[:]],
            outs=[rs_out[:]],
        )
        fin_bf = sb.tile([128, TPC // 128, H], BF16, name="fin_bf")
        nc.sync.dma_start(out=fin_bf[:], in_=rs_out.rearrange("(t p) h -> p t h", p=128))
        fin_f = sb.tile([128, TPC // 128, H], FP32, name="fin_f")
        nc.vector.tensor_copy(fin_f[:], fin_bf[:])
        nc.sync.dma_start(
            out=out_ext.rearrange("(t p) h -> p t h", p=128), in_=fin_f[:]
        )

    if split:
        split_multiwait(nc)
    return nc


_NC_DBG_CACHE = {}


def get_nc_debug(split=True):
    if split not in _NC_DBG_CACHE:
        _NC_DBG_CACHE[split] = build_module(debug=True, split=split)
    return _NC_DBG_CACHE[split]


def host_prep(x, sg_w, su_w, sd_w, router_w, routing_bias, wg, wu, wd):
    """Build the 8 per-core input maps from full inputs (numpy only)."""
    x2 = np.ascontiguousarray(x.reshape(T, H), dtype=np.float32)
    x_rows = x2.astype(BF)
    xT = np.ascontiguousarray(x2.T).astype(BF)

    rwT = np.ascontiguousarray(router_w.T.astype(np.float32))  # [H, E]
    rwT_h = rwT.astype(BF)
    rwT_l = (rwT - rwT_h.astype(np.float32)).astype(BF)
    bias_bc = np.ascontiguousarray(
        np.broadcast_to(routing_bias.astype(np.float32), (128, E))
    )
    ut = np.triu(np.ones((128, 128), np.float32), 1).astype(BF)

    in_maps = []
    for c in range(NCORE):
        xl = np.ascontiguousarray(x2[c * TPC : (c + 1) * TPC].T)  # [H, TPC] fp32
        xl_h = xl.astype(BF)
        xl_l = (xl - xl_h.astype(np.float32)).astype(BF)
        m = {
            "x_rows": x_rows,
            "xT": xT,
            "xTl_h": xl_h,
            "xTl_l": xl_l,
            "rwT_h": rwT_h,
            "rwT_l": rwT_l,
            "bias_bc": bias_bc,
            "wgT": np.ascontiguousarray(wg[c].T).astype(BF),
            "wuT": np.ascontiguousarray(wu[c].T).astype(BF),
            "wdT": np.ascontiguousarray(wd[c].T).astype(BF),
            "sgT": np.ascontiguousarray(sg_w[c * ISL : (c + 1) * ISL].T).astype(BF),
            "suT": np.ascontiguousarray(su_w[c * ISL : (c + 1) * ISL].T).astype(BF),
            "sdT": np.ascontiguousarray(sd_w[:, c * ISL : (c + 1) * ISL].T).astype(BF),
            "cvec": np.full((128, 1), float(c), np.float32),
            "ut_ones": ut,
        }
        in_maps.append(m)
    return in_maps


_NC_CACHE = []


def get_nc():
    if not _NC_CACHE:
        _NC_CACHE.append(build_module())
    return _NC_CACHE[0]


def run(in_maps, trace=False, **kw):
    from concourse.bass_utils import run_bass_kernel_spmd

    nc = get_nc()
    return run_bass_kernel_spmd(nc, in_maps, list(range(NCORE)), trace=trace, **kw)


def kernel(**inputs):
    orig_shape = inputs["x"].shape
    in_maps = host_prep(**{k: np.asarray(v) for k, v in inputs.items()})
    res = run(in_maps)
    out = np.concatenate([res.results[c]["out"] for c in range(NCORE)], axis=0)
    return out.reshape(orig_shape).astype(np.float32)


# revision 20
# speedup vs baseline: 1.0237x; 1.0237x over previous
"""DeepSeekMoE (T=4096, H=1024, I=2048, E=8 routed top-2 + 1 shared) on 8 TRN2 NeuronCores.

Strategy (expert-parallel + token-parallel hybrid):
  - Each core c owns routed expert c (weights sharded over cores) and owns
    tokens [c*512, (c+1)*512) for the shared expert and the final output.
  - Router runs data-parallel (each core routes its 512 tokens, exact-fp32 via
    bf16 hi/lo 3-product matmuls), results AllGather'd (tiny).
  - Each core compacts the token list routed to its expert (prefix-scan +
    triangular-ones matmul + indirect-DMA scatter), gathers those token rows,
    runs the expert MLP on a fixed-capacity batch, scales rows by their gates
    and writes the compact result Y_c [CAP, H].
  - AllGather(Y) -> every core indirect-gathers the two expert contributions
    for each of its own 512 tokens (positions recomputed locally from the
    replicated routing info) and adds them onto its shared-expert output.

All MLP matmuls run in bf16 (fp32 PSUM accumulation); the router is exact to
fp32 working precision so top-2 selection matches the fp32 reference.
"""

from contextlib import ExitStack

import numpy as np
import ml_dtypes

import concourse.bass as bass
import concourse.mybir as mybir
from concourse.tile import TileContext
from concourse.masks import make_identity

BF = ml_dtypes.bfloat16

T = 4096          # tokens
H = 1024          # hidden
I = 2048          # intermediate
E = 8             # routed experts
NCORE = 8
TPC = T // NCORE  # tokens per core (512)
CAP = 1152        # per-expert token capacity (seed-0 max count is 1076)
NTT = TPC // 128  # local token tiles (4)
NHB = H // 128    # hidden 128-blocks (8)
NIT = I // 128    # intermediate 128-blocks (16)
NCT = CAP // 128  # capacity tiles (9)
NJ = NCORE * NTT  # routing-grid columns; col j=(r*4+tt), token=512*(j//4)+128*(j%4)+p
BIGPOS = 60000.0  # out-of-bounds scatter position for unassigned tokens

FP32 = mybir.dt.float32
BF16 = mybir.dt.bfloat16
I32 = mybir.dt.int32
U32 = mybir.dt.uint32


def ts(i, s):
    return slice(i * s, (i + 1) * s)


def split_multiwait(nc, max_waits=1):
    """This container's walrus build rejects instructions carrying more than
    one fused semaphore wait ("Too many sync wait commands"). Offload extra
    waits onto standalone EventSemaphore instructions ahead of the owner —
    identical semantics (the sequencer blocks either way)."""
    n_split = 0
    for fn in nc.m.functions:
        for blk in fn.blocks:
            out = []
            for ins in blk.instructions:
                si = ins.sync_info
                if si is not None and si.on_wait and len(si.on_wait) > max_waits:
                    waits = list(si.on_wait)
                    for i, w in enumerate(waits[max_waits:]):
                        ev = mybir.InstEventSemaphore(
                            name=f"{ins.name}-evw{i}",
                            engine=ins.engine,
                            sync_info=mybir.SyncInfo(on_wait=[w], on_update=[]),
                        )
                        out.append(ev)
                        n_split += 1
                    si.on_wait = waits[:max_waits]
                out.append(ins)
            blk.instructions = out
    return n_split


def build_module(debug=False, split=True, hw_silu=True):
    nc = bass.Bass(num_devices=NCORE)

    def inp(name, shape, dtype):
        return nc.declare_dram_parameter(name, list(shape), dtype, isOutput=False)

    x_rows = inp("x_rows", (T, H), BF16)          # token-major x (gather source)
    xTl_h = inp("xTl_h", (H, TPC), BF16)          # local x.T hi (router lhsT + shared rhs)
    xTl_l = inp("xTl_l", (H, TPC), BF16)          # local x.T lo
    rwT_h = inp("rwT_h", (H, E), BF16)            # router w.T hi
    rwT_l = inp("rwT_l", (H, E), BF16)
    bias_bc = inp("bias_bc", (128, E), FP32)      # routing bias broadcast to 128 rows
    wgT = inp("wgT", (H, I), BF16)                # this core's expert gate w.T
    wuT = inp("wuT", (H, I), BF16)
    wdT = inp("wdT", (I, H), BF16)
    sgT = inp("sgT", (H, I), BF16)                # shared gate w.T (full)
    suT = inp("suT", (H, I), BF16)
    sdT = inp("sdT", (I, H), BF16)                # shared down w.T (full)
    cvec = inp("cvec", (128, 1), FP32)            # core id replicated
    ut_ones = inp("ut_ones", (128, 128), BF16)    # strict upper-triangular ones

    out_ext = nc.declare_dram_parameter("out", [TPC, H], FP32, isOutput=True)
    if debug:
        dbg_rt = nc.declare_dram_parameter("dbg_rt", [NCORE, 128, 16], FP32, isOutput=True)
        dbg_cmp = nc.declare_dram_parameter("dbg_cmp", [CAP, 2], FP32, isOutput=True)
        dbg_pos = nc.declare_dram_parameter("dbg_pos", [128, 2 * NTT], FP32, isOutput=True)
        dbg_y = nc.declare_dram_parameter("dbg_y", [CAP, H], BF16, isOutput=True)

    ACT_SILU = (
        mybir.ActivationFunctionType.Silu if hw_silu
        else mybir.ActivationFunctionType.Sigmoid
    )

    with TileContext(nc) as tc, ExitStack() as ctx:
        sb = ctx.enter_context(tc.tile_pool(name="sb", bufs=1))
        sb2 = ctx.enter_context(tc.tile_pool(name="sb2", bufs=2))
        ps_big = ctx.enter_context(tc.tile_pool(name="ps_big", bufs=4, space="PSUM"))
        ps_sm = ctx.enter_context(tc.tile_pool(name="ps_sm", bufs=2, space="PSUM"))
        dram = ctx.enter_context(tc.tile_pool(name="dram", bufs=1, space="DRAM"))

        ident = sb.tile([128, 128], BF16, name="ident")
        make_identity(nc, ident[:])

        def act_mul(out_ap, ps_g_ap, ps_u_ap, sil_tile):
            """out = silu(ps_g) * ps_u (all [128, n])."""
            nc.scalar.activation(sil_tile, ps_g_ap, ACT_SILU)
            if not hw_silu:
                nc.vector.tensor_mul(out=sil_tile, in0=sil_tile, in1=ps_g_ap)
            nc.vector.tensor_mul(out=out_ap, in0=sil_tile, in1=ps_u_ap)

        # ------------------------------------------------------------------
        # Phase R: router on local 512 tokens (exact via bf16 hi/lo products).
        # ------------------------------------------------------------------
        xtlh_sb = sb.tile([128, NHB, TPC], BF16, name="xtlh_sb")
        xtll_sb, xtll_free = tc.tile([128, NHB, TPC], BF16, name="xtll_sb")
        rwh_sb = sb.tile([128, NHB, E], BF16, name="rwh_sb")
        rwl_sb = sb.tile([128, NHB, E], BF16, name="rwl_sb")
        bias_sb = sb.tile([128, E], FP32, name="bias_sb")
        nc.sync.dma_start(out=xtlh_sb[:], in_=xTl_h.rearrange("(b p) t -> p b t", p=128))
        nc.sync.dma_start(out=xtll_sb[:], in_=xTl_l.rearrange("(b p) t -> p b t", p=128))
        nc.sync.dma_start(out=rwh_sb[:], in_=rwT_h.rearrange("(b p) e -> p b e", p=128))
        nc.sync.dma_start(out=rwl_sb[:], in_=rwT_l.rearrange("(b p) e -> p b e", p=128))
        nc.sync.dma_start(out=bias_sb[:], in_=bias_bc[:])

        rtloc = sb.tile([128, NTT, 4], FP32, name="rtloc")  # (i1, i2, g1, g2)
        for tt in range(NTT):
            ps_r = ps_sm.tile([128, E], FP32, name="ps_r", tag="ps_sm")
            pairs = [(xtlh_sb, rwh_sb), (xtlh_sb, rwl_sb), (xtll_sb, rwh_sb)]
            k, nmm = 0, len(pairs) * NHB
            for xs, ws in pairs:
                for hb in range(NHB):
                    nc.tensor.matmul(
                        out=ps_r[:], lhsT=xs[:, hb, ts(tt, 128)], rhs=ws[:, hb, :],
                        start=(k == 0), stop=(k == nmm - 1),
                    )
                    k += 1
            logit = sb2.tile([128, E], FP32, name="logit")
            nc.vector.tensor_add(out=logit[:], in0=ps_r[:], in1=bias_sb[:])
            vals = sb2.tile([128, 8], FP32, name="vals")
            idxs = sb2.tile([128, 8], U32, name="idxs")
            nc.vector.max(out=vals[:], in_=logit[:])
            nc.vector.max_index(out=idxs[:], in_max=vals[:], in_values=logit[:])
            p12 = sb2.tile([128, 2], FP32, name="p12")
            nc.scalar.activation(p12[:], vals[:, 0:2], mybir.ActivationFunctionType.Sigmoid)
            psum12 = sb2.tile([128, 1], FP32, name="psum12")
            nc.vector.tensor_add(out=psum12[:], in0=p12[:, 0:1], in1=p12[:, 1:2])
            rinv = sb2.tile([128, 1], FP32, name="rinv")
            nc.vector.reciprocal(out=rinv[:], in_=psum12[:])
            nc.vector.tensor_copy(rtloc[:, tt, 0:2], idxs[:, 0:2])
            nc.vector.tensor_scalar_mul(rtloc[:, tt, 2:4], p12[:], rinv[:])

        rt_local = dram.tile([128, NTT * 4], FP32, name="rt_local")
        rt_all = dram.tile([NCORE, 128, NTT * 4], FP32, name="rt_all", addr_space="Shared")
        nc.sync.dma_start(out=rt_local[:], in_=rtloc[:].rearrange("p t f -> p (t f)"))
        nc.gpsimd.collective_compute(
            "AllGather", mybir.AluOpType.bypass,
            replica_groups=[list(range(NCORE))],
            ins=[rt_local[:]], outs=[rt_all[:]],
        )

        # ------------------------------------------------------------------
        # Phase S1: shared expert gate/up on the local 512 tokens.
        # ------------------------------------------------------------------
        fin = sb.tile([128, NTT, H], FP32, name="fin")
        hts, hts_free = tc.tile([128, NIT, TPC], BF16, name="hts")
        for it in range(NIT):
            sg_sb = sb2.tile([128, NHB, 128], BF16, name="sg_sb", tag="sg_sb")
            su_sb = sb2.tile([128, NHB, 128], BF16, name="su_sb", tag="su_sb")
            nc.sync.dma_start(
                out=sg_sb[:], in_=sgT[:, ts(it, 128)].rearrange("(b p) i -> p b i", p=128)
            )
            nc.sync.dma_start(
                out=su_sb[:], in_=suT[:, ts(it, 128)].rearrange("(b p) i -> p b i", p=128)
            )
            ps_g = ps_big.tile([128, 512], FP32, name="ps_g", tag="ps_big")
            ps_u = ps_big.tile([128, 512], FP32, name="ps_u", tag="ps_big")
            for hb in range(NHB):
                nc.tensor.matmul(
                    out=ps_g[:], lhsT=sg_sb[:, hb, :], rhs=xtlh_sb[:, hb, :],
                    start=(hb == 0), stop=(hb == NHB - 1),
                )
            for hb in range(NHB):
                nc.tensor.matmul(
                    out=ps_u[:], lhsT=su_sb[:, hb, :], rhs=xtlh_sb[:, hb, :],
                    start=(hb == 0), stop=(hb == NHB - 1),
                )
            sil = sb2.tile([128, 512], FP32, name="sil", tag="sil")
            act_mul(hts[:, it, :], ps_g[:], ps_u[:], sil[:])

        # ------------------------------------------------------------------
        # Phase C: routing bookkeeping over all T tokens (after AllGather).
        # ------------------------------------------------------------------
        rt_sb = sb.tile([128, NJ, 4], FP32, name="rt_sb")
        nc.sync.dma_start(
            out=rt_sb[:].rearrange("p (r t) f -> p r t f", r=NCORE),
            in_=rt_all.rearrange("r p (t f) -> p r t f", f=4),
        )
        cvec_sb = sb.tile([128, 1], FP32, name="cvec_sb")
        nc.sync.dma_start(out=cvec_sb[:], in_=cvec[:])
        ut_sb = sb.tile([128, 128], BF16, name="ut_sb")
        nc.sync.dma_start(out=ut_sb[:], in_=ut_ones[:])
        zeros = sb.tile([128, NJ], FP32, name="zeros")
        nc.vector.memset(zeros[:], 0.0)
        tok_iota = sb.tile([128, NCORE, NTT], I32, name="tok_iota")
        nc.gpsimd.iota(tok_iota[:], pattern=[[TPC, NCORE], [128, NTT]], base=0,
                       channel_multiplier=1)
        iota_e = sb.tile([128, E], I32, name="iota_e")
        nc.gpsimd.iota(iota_e[:], pattern=[[1, E]], base=0, channel_multiplier=0)
        iota_ef = sb.tile([128, E], FP32, name="iota_ef")
        nc.vector.tensor_copy(iota_ef[:], iota_e[:])
        onehot = sb.tile([128, E], FP32, name="onehot")
        nc.vector.tensor_scalar(onehot[:], iota_ef[:], cvec_sb[:], None,
                                op0=mybir.AluOpType.is_equal)

        # per-expert membership masks + exclusive-prefix positions (all tokens)
        m1_keep = sb.tile([128, E, NJ], FP32, name="m1_keep")
        m2_keep = sb.tile([128, E, NJ], FP32, name="m2_keep")
        pose_keep = sb.tile([128, E, NJ], FP32, name="pose_keep")
        idx1, idx2 = rt_sb[:, :, 0], rt_sb[:, :, 1]
        for e in range(E):
            m1e, m2e, pose = m1_keep[:, e, :], m2_keep[:, e, :], pose_keep[:, e, :]
            nc.vector.tensor_scalar(m1e, idx1, float(e), None, op0=mybir.AluOpType.is_equal)
            nc.vector.tensor_scalar(m2e, idx2, float(e), None, op0=mybir.AluOpType.is_equal)
            maske = sb2.tile([128, NJ], FP32, name="maske", tag="maske")
            nc.vector.tensor_add(out=maske[:], in0=m1e, in1=m2e)
            nc.vector.tensor_tensor_scan(
                out=pose, data0=maske[:], data1=zeros[:], initial=0.0,
                op0=mybir.AluOpType.add, op1=mybir.AluOpType.add,
            )
            rowtot_bf = sb2.tile([128, 1], BF16, name="rowtot_bf", tag="rowtot_bf")
            nc.vector.tensor_copy(rowtot_bf[:], pose_keep[:, e, NJ - 1 : NJ])
            ps_cum = ps_sm.tile([128, 1], FP32, name="ps_cum", tag="ps_sm")
            nc.tensor.matmul(out=ps_cum[:], lhsT=ut_sb[:], rhs=rowtot_bf[:],
                             start=True, stop=True)
            base = sb2.tile([128, 1], FP32, name="base", tag="base")
            nc.vector.tensor_copy(base[:], ps_cum[:])
            nc.vector.tensor_sub(out=pose, in0=pose, in1=maske[:])
            nc.vector.tensor_scalar_add(pose, pose, base[:])

        # our expert's masks/positions/gates (select e == c via one-hot)
        m1c = sb.tile([128, NJ], FP32, name="m1c")
        m2c = sb.tile([128, NJ], FP32, name="m2c")
        posc = sb.tile([128, NJ], FP32, name="posc")
        nc.vector.memset(m1c[:], 0.0)
        nc.vector.memset(m2c[:], 0.0)
        nc.vector.memset(posc[:], 0.0)
        for e in range(E):
            t1 = sb2.tile([128, NJ], FP32, name="t1", tag="t1")
            nc.vector.tensor_scalar_mul(t1[:], m1_keep[:, e, :], onehot[:, e : e + 1])
            nc.vector.tensor_add(out=m1c[:], in0=m1c[:], in1=t1[:])
            nc.vector.tensor_scalar_mul(t1[:], m2_keep[:, e, :], onehot[:, e : e + 1])
            nc.vector.tensor_add(out=m2c[:], in0=m2c[:], in1=t1[:])
            nc.vector.tensor_scalar_mul(t1[:], pose_keep[:, e, :], onehot[:, e : e + 1])
            nc.vector.tensor_add(out=posc[:], in0=posc[:], in1=t1[:])
        maskc = sb.tile([128, NJ], FP32, name="maskc")
        gatec = sb.tile([128, NJ], FP32, name="gatec")
        nc.vector.tensor_add(out=maskc[:], in0=m1c[:], in1=m2c[:])
        t2 = sb.tile([128, NJ], FP32, name="t2")
        nc.vector.tensor_mul(out=t2[:], in0=m1c[:], in1=rt_sb[:, :, 2])
        nc.vector.tensor_mul(out=gatec[:], in0=m2c[:], in1=rt_sb[:, :, 3])
        nc.vector.tensor_add(out=gatec[:], in0=gatec[:], in1=t2[:])
        notm = sb.tile([128, NJ], FP32, name="notm")
        nc.vector.tensor_scalar(notm[:], maskc[:], -BIGPOS, BIGPOS,
                                op0=mybir.AluOpType.mult, op1=mybir.AluOpType.add)
        nc.vector.tensor_add(out=posc[:], in0=posc[:], in1=notm[:])
        upos = sb.tile([128, NJ], I32, name="upos")
        nc.vector.tensor_copy(upos[:], posc[:])

        rec = sb.tile([128, NJ, 2], FP32, name="rec")
        nc.vector.tensor_copy(rec[:, :, 0], tok_iota[:].rearrange("p a b -> p (a b)"))
        nc.vector.tensor_copy(rec[:, :, 1], gatec[:])

        cmp_d = dram.tile([CAP, 2], FP32, name="cmp_d")
        zrow = sb.tile([128, CAP * 2 // 128], FP32, name="zrow")
        nc.vector.memset(zrow[:], 0.0)
        nc.sync.dma_start(out=cmp_d.rearrange("(p t) f -> p (t f)", p=128), in_=zrow[:])
        # HW indirect DMA honors one offset per partition: scatter column-wise.
        for j in range(NJ):
            nc.gpsimd.indirect_dma_start(
                out=cmp_d[:],
                out_offset=bass.IndirectOffsetOnAxis(ap=upos[:, j : j + 1], axis=0),
                in_=rec[:, j, :],
                in_offset=None,
                bounds_check=CAP - 1,
                oob_is_err=False,
            )
        # read back compact list: slot s = t*128 + p  ->  [p, t]
        cmp_sb = sb.tile([128, NCT, 2], FP32, name="cmp_sb")
        nc.sync.dma_start(out=cmp_sb[:], in_=cmp_d.rearrange("(t p) f -> p t f", p=128))
        tok_i = sb.tile([128, NCT], I32, name="tok_i")
        nc.vector.tensor_copy(tok_i[:], cmp_sb[:, :, 0])

        # phase-F gather indices for the LOCAL tokens: global slot id
        # e*CAP + pos_e(t) for slot-1/2 experts; local token (p, tt) is
        # column j = c*NTT + tt (select r == c via one-hot over r).
        pos_sel = sb.tile([128, NTT, 2], FP32, name="pos_sel")
        for sl, mkeep in ((0, m1_keep), (1, m2_keep)):
            fld = sb2.tile([128, NJ], FP32, name="fld", tag="fld")
            nc.vector.memset(fld[:], 0.0)
            for e in range(E):
                te = sb2.tile([128, NJ], FP32, name="te", tag="te")
                nc.vector.tensor_scalar(te[:], pose_keep[:, e, :], float(e * CAP), None,
                                        op0=mybir.AluOpType.add)
                nc.vector.tensor_mul(out=te[:], in0=te[:], in1=mkeep[:, e, :])
                nc.vector.tensor_add(out=fld[:], in0=fld[:], in1=te[:])
            acc = sb2.tile([128, NTT], FP32, name="acc", tag="acc")
            nc.vector.memset(acc[:], 0.0)
            fv = fld[:].rearrange("p (r t) -> p r t", r=NCORE)
            for r in range(NCORE):
                t3 = sb2.tile([128, NTT], FP32, name="t3", tag="t3")
                nc.vector.tensor_scalar_mul(t3[:], fv[:, r, :], onehot[:, r : r + 1])
                nc.vector.tensor_add(out=acc[:], in0=acc[:], in1=t3[:])
            nc.vector.tensor_copy(pos_sel[:, :, sl], acc[:])
        posl_i = sb.tile([128, NTT, 2], I32, name="posl_i")
        nc.vector.tensor_copy(posl_i[:], pos_sel[:])

        # ------------------------------------------------------------------
        # Phase G: gather + transpose this expert's token rows -> xgT [H, CAP].
        # ------------------------------------------------------------------
        xgT = sb.tile([128, NHB, CAP], BF16, name="xgT")
        for ct in range(NCT):
            xg = sb2.tile([128, H], BF16, name="xg", tag="xg")
            nc.gpsimd.indirect_dma_start(
                out=xg[:],
                out_offset=None,
                in_=x_rows[:],
                in_offset=bass.IndirectOffsetOnAxis(ap=tok_i[:, ct : ct + 1], axis=0),
            )
            for hb in range(NHB):
                ps_t = ps_sm.tile([128, 128], BF16, name="ps_t", tag="ps_sm")
                nc.tensor.transpose(out=ps_t[:], in_=xg[:, ts(hb, 128)], identity=ident[:])
                nc.vector.tensor_copy(xgT[:, hb, ts(ct, 128)], ps_t[:])

        # ------------------------------------------------------------------
        # Phase S2: shared expert down-projection -> fin (fp32, SBUF).
        # ------------------------------------------------------------------
        sd_sb, sd_free = tc.tile([128, NIT, H], BF16, name="sd_sb")
        nc.sync.dma_start(out=sd_sb[:], in_=sdT.rearrange("(b p) h -> p b h", p=128))
        for mt in range(NTT):
            for nch in range(H // 512):
                ps_d = ps_big.tile([128, 512], FP32, name="ps_d", tag="ps_big")
                for it in range(NIT):
                    nc.tensor.matmul(
                        out=ps_d[:],
                        lhsT=hts[:, it, ts(mt, 128)],
                        rhs=sd_sb[:, it, ts(nch, 512)],
                        start=(it == 0),
                        stop=(it == NIT - 1),
                    )
                nc.vector.tensor_copy(fin[:, mt, ts(nch, 512)], ps_d[:])
        sd_free()
        hts_free()
        xtll_free()

        # ------------------------------------------------------------------
        # Phase E: routed expert MLP on the capacity batch -> Y_c (gate-scaled).
        # ------------------------------------------------------------------
        hT = sb.tile([128, NIT, CAP], BF16, name="hT")
        ECH = [(0, 512), (512, 512), (1024, CAP - 1024)]
        for it in range(NIT):
            wg_sb = sb2.tile([128, NHB, 128], BF16, name="wg_sb", tag="wg_sb")
            wu_sb = sb2.tile([128, NHB, 128], BF16, name="wu_sb", tag="wu_sb")
            nc.sync.dma_start(
                out=wg_sb[:], in_=wgT[:, ts(it, 128)].rearrange("(b p) i -> p b i", p=128)
            )
            nc.sync.dma_start(
                out=wu_sb[:], in_=wuT[:, ts(it, 128)].rearrange("(b p) i -> p b i", p=128)
            )
            for c0, cn in ECH:
                ps_g = ps_big.tile([128, 512], FP32, name="ps_g", tag="ps_big")
                ps_u = ps_big.tile([128, 512], FP32, name="ps_u", tag="ps_big")
                for hb in range(NHB):
                    nc.tensor.matmul(
                        out=ps_g[:, :cn], lhsT=wg_sb[:, hb, :], rhs=xgT[:, hb, c0 : c0 + cn],
                        start=(hb == 0), stop=(hb == NHB - 1),
                    )
                for hb in range(NHB):
                    nc.tensor.matmul(
                        out=ps_u[:, :cn], lhsT=wu_sb[:, hb, :], rhs=xgT[:, hb, c0 : c0 + cn],
                        start=(hb == 0), stop=(hb == NHB - 1),
                    )
                sil = sb2.tile([128, 512], FP32, name="sil", tag="sil")
                act_mul(hT[:, it, c0 : c0 + cn], ps_g[:, :cn], ps_u[:, :cn], sil[:, :cn])

        wd_sb = sb.tile([128, NIT, H], BF16, name="wd_sb")
        nc.sync.dma_start(out=wd_sb[:], in_=wdT.rearrange("(b p) h -> p b h", p=128))

        y_c = dram.tile([CAP, H], BF16, name="y_c")
        y_all = dram.tile([NCORE, CAP, H], BF16, name="y_all", addr_space="Shared")
        for ct in range(NCT):
            yrow = sb2.tile([128, H], BF16, name="yrow", tag="yrow")
            for nch in range(H // 512):
                ps_d = ps_big.tile([128, 512], FP32, name="ps_d", tag="ps_big")
                for it in range(NIT):
                    nc.tensor.matmul(
                        out=ps_d[:],
                        lhsT=hT[:, it, ts(ct, 128)],
                        rhs=wd_sb[:, it, ts(nch, 512)],
                        start=(it == 0),
                        stop=(it == NIT - 1),
                    )
                nc.vector.tensor_scalar_mul(yrow[:, ts(nch, 512)], ps_d[:], cmp_sb[:, ct, 1:2])
            nc.sync.dma_start(out=y_c[ts(ct, 128), :], in_=yrow[:])

        nc.gpsimd.collective_compute(
            "AllGather", mybir.AluOpType.bypass,
            replica_groups=[list(range(NCORE))],
            ins=[y_c[:]], outs=[y_all[:]],
        )

        # ------------------------------------------------------------------
        # Phase F: combine — gather both expert contributions for the local
        # tokens from y_all, add onto the shared output, emit fp32.
        # ------------------------------------------------------------------
        y_flat = y_all.rearrange("e c h -> (e c) h")
        for mt in range(NTT):
            for sl in range(2):
                yg = sb2.tile([128, H], BF16, name="yg", tag="yg")
                nc.gpsimd.indirect_dma_start(
                    out=yg[:],
                    out_offset=None,
                    in_=y_flat,
                    in_offset=bass.IndirectOffsetOnAxis(
                        ap=posl_i[:, mt, sl : sl + 1], axis=0
                    ),
                )
                nc.vector.tensor_add(out=fin[:, mt, :], in0=fin[:, mt, :], in1=yg[:])
            nc.sync.dma_start(out=out_ext[ts(mt, 128), :], in_=fin[:, mt, :])

        if debug:
            nc.sync.dma_start(out=dbg_rt[:], in_=rt_all[:])
            nc.sync.dma_start(out=dbg_cmp[:], in_=cmp_d[:])
            nc.sync.dma_start(out=dbg_pos[:], in_=pos_sel[:].rearrange("p t f -> p (t f)"))
            nc.sync.dma_start(out=dbg_y[:], in_=y_c[:])

    if split:
        split_multiwait(nc)
    return nc


def host_prep(x, sg_w, su_w, sd_w, router_w, routing_bias, wg, wu, wd):
    """Build the 8 per-core input maps from full inputs (numpy only)."""
    x2 = np.ascontiguousarray(x.reshape(T, H), dtype=np.float32)
    x_rows = x2.astype(BF)

    rwT = np.ascontiguousarray(router_w.T.astype(np.float32))  # [H, E]
    rwT_h = rwT.astype(BF)
    rwT_l = (rwT - rwT_h.astype(np.float32)).astype(BF)
    bias_bc = np.ascontiguousarray(
        np.broadcast_to(routing_bias.astype(np.float32), (128, E))
    )
    ut = np.triu(np.ones((128, 128), np.float32), 1).astype(BF)
    sgT = np.ascontiguousarray(sg_w.T).astype(BF)
    suT = np.ascontiguousarray(su_w.T).astype(BF)
    sdT = np.ascontiguousarray(sd_w.T).astype(BF)

    in_maps = []
    for c in range(NCORE):
        xl = np.ascontiguousarray(x2[c * TPC : (c + 1) * TPC].T)  # [H, TPC] fp32
        xl_h = xl.astype(BF)
        xl_l = (xl - xl_h.astype(np.float32)).astype(BF)
        m = {
            "x_rows": x_rows,
            "xTl_h": xl_h,
            "xTl_l": xl_l,
            "rwT_h": rwT_h,
            "rwT_l": rwT_l,
            "bias_bc": bias_bc,
            "wgT": np.ascontiguousarray(wg[c].T).astype(BF),
            "wuT": np.ascontiguousarray(wu[c].T).astype(BF),
            "wdT": np.ascontiguousarray(wd[c].T).astype(BF),
            "sgT": sgT,
            "suT": suT,
            "sdT": sdT,
            "cvec": np.full((128, 1), float(c), np.float32),
            "ut_ones": ut,
        }
        in_maps.append(m)
    return in_maps


_NC_CACHE = {}


def get_nc(debug=False, split=True, hw_silu=True):
    key = (debug, split, hw_silu)
    if key not in _NC_CACHE:
        _NC_CACHE[key] = build_module(debug=debug, split=split, hw_silu=hw_silu)
    return _NC_CACHE[key]


def get_nc_debug(split=True, hw_silu=True):
    return get_nc(debug=True, split=split, hw_silu=hw_silu)


def run(in_maps, trace=False, **kw):
    from concourse.bass_utils import run_bass_kernel_spmd

    nc = get_nc()
    return run_bass_kernel_spmd(nc, in_maps, list(range(NCORE)), trace=trace, **kw)


def kernel(**inputs):
    orig_shape = inputs["x"].shape
    in_maps = host_prep(**{k: np.asarray(v) for k, v in inputs.items()})
    res = run(in_maps)
    out = np.concatenate([res.results[c]["out"] for c in range(NCORE)], axis=0)
    return out.reshape(orig_shape).astype(np.float32)
